# revision 19
# baseline (speedup 1.0000x reference)
# Causal multi-head attention (B=4, L=2048, H=16, E=64, fp32) on 8 TRN2
# NeuronCores. Sharding: the 64 (b,h) pairs split 8 per core; each core
# computes its pairs fully independently (data parallel on B, tensor
# parallel on H).
#
# Per-core algorithm (heads processed two at a time, packed into the two
# 64-row halves of the PE array for the score matmuls):
#   Q,K,V arrive in SBUF as bf16 via casting gpsimd DMAs (no cast ops)
#   Q,K are PE-transposed (both heads per instruction) to [e, l] layout
#   S^T[s,l] = K^T . Q  chunks in PSUM (causal-skipped, bf16 matmul)
#   P^T = exp(S^T/8)    split between ScalarE (exact exp) and VectorE
#   (Schraudolph fast-exp: bf16 bit pattern built via int16 convert)
#   diagonal tiles masked by an upper-triangular 0/1 multiply on VectorE
#   O^T[d,l] accumulates in PSUM with V (ones-augmented) stationary and
#   P^T streaming; AV matmuls trail the score matmuls by 3 chunks
#   O^T is copied to SBUF as bf16 on GpSimd, PE-transposed back (bf16),
#   and divided by the rowsum row on VectorE; one store DMA per pair.
# L is processed in 4 phases of 512 columns to bound SBUF; pair prologues
# are software-pipelined into the previous pair's later phases.

import sys

import numpy as np

try:
    import concourse.bass as bass  # noqa: F401
except ImportError:
    sys.path.insert(0, "/opt/trn_rl_repo")

B, L, H, E = 4, 2048, 16, 64
NCORES = 8
BH = B * H                  # 64 (b,h) pairs
BH_PER_CORE = BH // NCORES  # 8
NPAIRS = BH_PER_CORE // 2   # 4 packed pairs per core
NLT = L // 128              # 16 l-tiles
NPH = 4                     # phases over l
PHL = L // NPH              # 512 l-cols per phase
VW = 66                     # V columns + ones col + zero pad

# Schraudolph fast-exp constants for bf16 output:
#   bits_i16 = round((S * scale) * log2(e) * 128 + (127*128 - 128*c))
# with c = 0.0436775 balancing the max relative error to ~±3%.
EXP_A = (1.0 / 8.0) * 1.4426950408889634 * 128.0   # 23.08312...
EXP_B = 127.0 * 128.0 - 5.5907                      # 16250.41

_CACHE = {}


def _phase_chunks(ph):
    """(st, lstart, w) for every s-tile contributing to phase ph."""
    lo, hi = ph * PHL, (ph + 1) * PHL
    return [(st, max(st * 128, lo), hi - max(st * 128, lo)) for st in range(4 * ph + 4)]


def _build_program():
    from contextlib import ExitStack

    import concourse.bass as bass
    import concourse.mybir as mybir
    import concourse.tile as tile
    from concourse import bacc
    from concourse.masks import make_identity, make_upper_triangular

    f32 = mybir.dt.float32
    bf16 = mybir.dt.bfloat16
    i16 = mybir.dt.int16

    nc = bacc.Bacc(
        "TRN2",
        target_bir_lowering=False,
        debug=False,
        enable_asserts=False,
        num_devices=NCORES,
    )
    q_d = nc.dram_tensor("q", [BH_PER_CORE, L, E], f32, kind="ExternalInput").ap()
    k_d = nc.dram_tensor("k", [BH_PER_CORE, L, E], f32, kind="ExternalInput").ap()
    v_d = nc.dram_tensor("v", [BH_PER_CORE, L, E], f32, kind="ExternalInput").ap()
    o_d = nc.dram_tensor("o", [BH_PER_CORE, L, E], f32, kind="ExternalOutput").ap()

    with tile.TileContext(nc) as tc, ExitStack() as ctx:
        consts = ctx.enter_context(tc.tile_pool(name="consts", bufs=1))
        stage = ctx.enter_context(tc.tile_pool(name="stage", bufs=2))
        qkt = ctx.enter_context(tc.tile_pool(name="qkt", bufs=8))
        ptc = ctx.enter_context(tc.tile_pool(name="ptc", bufs=20))
        otsbp = ctx.enter_context(tc.tile_pool(name="otsbp", bufs=2))
        outp = ctx.enter_context(tc.tile_pool(name="outp", bufs=2))
        recp = ctx.enter_context(tc.tile_pool(name="recp", bufs=4))
        spsum = ctx.enter_context(tc.tile_pool(name="spsum", bufs=3, space="PSUM"))
        otps = ctx.enter_context(tc.tile_pool(name="otps", bufs=2, space="PSUM"))

        ident = consts.tile([128, 128], bf16)
        make_identity(nc, ident)
        identf = consts.tile([128, 128], f32)
        make_identity(nc, identf)
        # mask01[s, j] = 1.0 where s <= j else 0.0 (valid causal region of a
        # diagonal tile of P^T)
        mask01 = consts.tile([128, 128], bf16)
        make_upper_triangular(nc, mask01, val=1.0, diag=True)
        mask01_ap = mask01[:]
        mask01_b = bass.AP(
            tensor=mask01_ap.tensor,
            offset=mask01_ap.offset,
            ap=[mask01_ap.ap[0], [0, 2], mask01_ap.ap[1]],
        )

        scale = 1.0 / float(np.sqrt(E))

        state = {}

        def prologue_load(p, fine=False):
            # bf16 staged tensors; the cast happens inside the gpsimd DMA.
            # qf/kf keep (j, e) contiguous so a single PE transpose covers
            # both heads of one l-tile with a one-free-dim stationary AP.
            qf = stage.tile([128, NLT, 2, E], bf16, tag="qf", name="qf")
            kf = stage.tile([128, NLT, 2, E], bf16, tag="kf", name="kf")
            vaug = stage.tile([128, 2, NLT, VW], bf16, tag="vaug", name="vaug")
            qr = [q_d[2 * p + j].rearrange("(t pp) e -> pp t e", pp=128) for j in range(2)]
            kr = [k_d[2 * p + j].rearrange("(t pp) e -> pp t e", pp=128) for j in range(2)]
            vr = [v_d[2 * p + j].rearrange("(t pp) e -> pp t e", pp=128) for j in range(2)]
            if fine:
                # first pair: the c=0 quarter of q,k goes over the two fast
                # HWDGE queues (sync + scalar) in fp32 so the first transposes
                # start ~5us earlier; everything else uses casting gpsimd DMAs
                c0 = slice(0, 4)
                rest = slice(4, NLT)
                qf32 = stage.tile([128, 4, 2, E], f32, tag="qf32", name="qf32")
                kf32 = stage.tile([128, 4, 2, E], f32, tag="kf32", name="kf32")
                for j in range(2):
                    nc.sync.dma_start(out=qf32[:, :, j], in_=qr[j][:, c0])
                    nc.scalar.dma_start(out=kf32[:, :, j], in_=kr[j][:, c0])
                for j in range(2):
                    nc.gpsimd.dma_start(out=vaug[:, j, c0, 0:E], in_=vr[j][:, c0])
                for j in range(2):
                    nc.gpsimd.dma_start(out=qf[:, rest, j], in_=qr[j][:, rest])
                    nc.gpsimd.dma_start(out=kf[:, rest, j], in_=kr[j][:, rest])
                for j in range(2):
                    nc.gpsimd.dma_start(out=vaug[:, j, rest, 0:E], in_=vr[j][:, rest])
                state[p] = dict(qf=qf, kf=kf, vaug=vaug, qf32=qf32, kf32=kf32)
                return
            else:
                # one casting DMA per tensor per head
                for j in range(2):
                    nc.gpsimd.dma_start(out=qf[:, :, j], in_=qr[j])
                    nc.gpsimd.dma_start(out=kf[:, :, j], in_=kr[j])
                for j in range(2):
                    nc.gpsimd.dma_start(out=vaug[:, j, :, 0:E], in_=vr[j])
            state[p] = dict(qf=qf, kf=kf, vaug=vaug)

        def prologue_compute_units(p):
            """Emit-callables for pair p's transpose prologue, split so they
            can be spread between score chunks of the previous pair."""
            s = state[p]
            qf, kf, vaug = s["qf"], s["kf"], s["vaug"]
            qtc = [qkt.tile([128, 512], bf16, tag="qt", name="qtc") for _ in range(4)]
            ktc = [qkt.tile([128, 512], bf16, tag="kt", name="ktc") for _ in range(4)]
            ob = outp.tile([128, 2, NLT, E], f32, tag="ob", name="ob")
            s.update(qtc=qtc, ktc=ktc, ob=ob)

            def memsets():
                nc.gpsimd.memset(vaug[:, :, :, E : E + 1], 1.0)
                nc.gpsimd.memset(vaug[:, :, :, E + 1 : VW], 0.0)

            def unit(c):
                # both heads of one l-tile transposed in a single instruction:
                # in_ [128, (j,e)=128] -> out [128 rows=(j,e), 128 l]
                if c == 0 and "qf32" in s:
                    # pair 0's first quarter arrived fp32 over the HWDGE
                    # queues; transpose fp32 and cast on the copy-out
                    tps = spsum.tile([128, 512], f32, tag="sp", name="tpsf")
                    tps2 = spsum.tile([128, 512], f32, tag="sp", name="tpsf2")
                    for i in range(4):
                        nc.tensor.transpose(
                            out=tps[:, 128 * i : 128 * (i + 1)],
                            in_=s["qf32"][:, i],
                            identity=identf,
                        )
                        nc.tensor.transpose(
                            out=tps2[:, 128 * i : 128 * (i + 1)],
                            in_=s["kf32"][:, i],
                            identity=identf,
                        )
                    nc.vector.tensor_copy(qtc[0], tps)
                    nc.vector.tensor_copy(ktc[0], tps2)
                    return
                tps = spsum.tile([128, 1024], bf16, tag="sp", name="tps")
                for i in range(4):
                    nc.tensor.transpose(
                        out=tps[:, 128 * i : 128 * (i + 1)],
                        in_=qf[:, 4 * c + i],
                        identity=ident,
                    )
                    nc.tensor.transpose(
                        out=tps[:, 512 + 128 * i : 512 + 128 * (i + 1)],
                        in_=kf[:, 4 * c + i],
                        identity=ident,
                    )
                nc.vector.tensor_copy(qtc[c], tps[:, 0:512])
                nc.vector.tensor_copy(ktc[c], tps[:, 512:1024])

            return [memsets] + [lambda c=c: unit(c) for c in range(4)]

        def phase(p, ph, fillers=None, cadence=4):
            s = state[p]
            vaug, qtc, ktc, ob = s["vaug"], s["qtc"], s["ktc"], s["ob"]
            lo = ph * PHL
            chunks = _phase_chunks(ph)
            nst = len(chunks)
            ots = [otps.tile([VW, PHL], f32, tag="ot", name="ot") for _ in range(2)]
            pts = {}

            def emit_av(idx):
                st, lstart, w = chunks[idx]
                pt = pts[st]
                for j in range(2):
                    nc.tensor.matmul(
                        out=ots[j][:, lstart - lo : lstart - lo + w],
                        lhsT=vaug[:, j, st, :],
                        rhs=pt[:, 512 * j : 512 * j + w],
                        start=(st == 0),
                        stop=(st == nst - 1),
                    )

            for idx, (st, lstart, w) in enumerate(chunks):
                s0 = st * 128
                sp = spsum.tile([128, 1024], f32, tag="sp", name="sp")
                for j in range(2):
                    nc.tensor.matmul(
                        out=sp[:, 512 * j : 512 * j + w],
                        lhsT=ktc[st // 4][64 * j : 64 * (j + 1), (s0 % 512) : (s0 % 512) + 128],
                        rhs=qtc[ph][64 * j : 64 * (j + 1), lstart - lo : lstart - lo + w],
                        start=True,
                        stop=True,
                    )
                pt = ptc.tile([128, 1024], bf16, tag="pt", name="pt")
                if idx % 5 < 3:
                    # split: exact exp on ScalarE for head 0, Schraudolph
                    # fast-exp on VectorE for head 1
                    nc.scalar.activation(
                        pt[:, 0:w], sp[:, 0:w],
                        mybir.ActivationFunctionType.Exp, scale=scale,
                    )
                    nc.vector.tensor_scalar(
                        pt[:, 512 : 512 + w].bitcast(i16),
                        sp[:, 512 : 512 + w],
                        EXP_A,
                        EXP_B,
                        mybir.AluOpType.mult,
                        mybir.AluOpType.add,
                    )
                else:
                    # both heads exact on ScalarE
                    sp_v = sp.rearrange("pp (j c) -> pp j c", j=2)[:, :, 0:w]
                    pt_v = pt.rearrange("pp (j c) -> pp j c", j=2)[:, :, 0:w]
                    nc.scalar.activation(
                        pt_v, sp_v, mybir.ActivationFunctionType.Exp, scale=scale
                    )
                if lstart == s0:
                    # diagonal tile: zero the s > l half (both heads at once).
                    # Keep this on VectorE: anything Pool-side can stall for
                    # microseconds behind SWDGE descriptor generation, and
                    # the AV matmuls (and thus the PE clock) gate on the mask
                    dv = pt.rearrange("pp (j c) -> pp j c", j=2)[:, :, 0:128]
                    nc.vector.tensor_mul(dv, dv, mask01_b)
                pts[st] = pt
                if idx >= 4:
                    emit_av(idx - 4)
                if fillers and idx % cadence == cadence - 1:
                    fillers.popleft()()
            for k in (4, 3, 2, 1):
                if nst - k >= 0:
                    emit_av(nst - k)

            # ---- O^T epilogue: bf16 copy on Pool, transpose back, divide
            # by the rowsum row ----
            for j in range(2):
                otsb = otsbp.tile([VW, PHL], bf16, tag="otsb", name="otsb")
                nc.vector.tensor_copy(otsb, ots[j])
                tr = otps.tile([128, 4, VW], bf16, tag="ot", name="tr")
                for i in range(4):
                    nc.tensor.transpose(
                        out=tr[:, i, :],
                        in_=otsb[:, 128 * i : 128 * (i + 1)],
                        identity=ident[0:VW, 0:VW],
                    )
                rc = recp.tile([128, 4], f32, tag="rc", name="rc")
                nc.vector.reciprocal(rc, tr[:, :, E])
                rc_ap = rc[:]
                rc_b = bass.AP(
                    tensor=rc_ap.tensor,
                    offset=rc_ap.offset,
                    ap=list(rc_ap.ap) + [[0, E]],
                )
                nc.vector.tensor_mul(
                    ob[:, j, 4 * ph : 4 * ph + 4, :], tr[:, :, 0:E], rc_b
                )

        from collections import deque

        def emit_store(p, sl=None):
            ob = state[p]["ob"]
            if sl is None:
                nc.sync.dma_start(
                    out=o_d[2 * p : 2 * p + 2].rearrange(
                        "j (t pp) e -> pp j t e", pp=128
                    ),
                    in_=ob,
                )
            else:
                for j in range(2):
                    nc.sync.dma_start(
                        out=o_d[2 * p + j].rearrange("(t pp) e -> pp t e", pp=128)[
                            :, sl
                        ],
                        in_=ob[:, j, sl],
                    )

        prologue_load(0, fine=True)
        # warm the PE HAM clock while the first loads are in flight: dummy
        # matmuls on a zeroed tile (NOT ident — identity generation sits
        # behind the Pool preamble, which would delay the ramp ~2.5us)
        wsrc = consts.tile([128, 128], bf16)
        nc.vector.memset(wsrc, 0.0)
        warm = spsum.tile([128, 1024], f32, tag="sp", name="warm")
        for _ in range(36):
            nc.tensor.matmul(
                out=warm[:, 0:128], lhsT=wsrc, rhs=wsrc, start=True, stop=True
            )
        warmsb = consts.tile([128, 8], f32)
        nc.vector.tensor_copy(warmsb, warm[:, 0:8])
        units0 = prologue_compute_units(0)
        units0[0]()  # memsets
        units0[1]()  # c=0 transposes
        fillers0 = deque(units0[2:])
        for p in range(NPAIRS):
            if p + 1 < NPAIRS:
                # issue next pair's loads first: the SWDGE descriptor grind
                # on Pool takes ~10us serial, so give it the whole pair
                prologue_load(p + 1)
                phase(p, 0, fillers0 if p == 0 else None, cadence=1)
                phase(p, 1, fillers0 if p == 0 else None, cadence=4)
                fillers = deque(prologue_compute_units(p + 1))
                phase(p, 2, fillers)
                phase(p, 3, fillers)
                while fillers:
                    fillers.popleft()()
                emit_store(p)
            else:
                # last pair: biggest phase first so the post-exp tail is
                # short; store each phase as soon as it completes
                for ph in (3, 2, 1, 0):
                    phase(p, ph)
                    emit_store(p, slice(4 * ph, 4 * ph + 4))
            del state[p]

    nc.compile()
    return nc


def _get_program():
    if "nc" not in _CACHE:
        _CACHE["nc"] = _build_program()
    return _CACHE["nc"]


def kernel(queries=None, keys=None, values=None, **kw):
    if queries is None or keys is None or values is None:
        raise TypeError("kernel expects queries, keys, values")
    from concourse.bass_utils import run_bass_kernel_spmd

    q = np.ascontiguousarray(np.asarray(queries, dtype=np.float32))
    k = np.ascontiguousarray(np.asarray(keys, dtype=np.float32))
    v = np.ascontiguousarray(np.asarray(values, dtype=np.float32))
    assert q.shape == (B, L, H, E), q.shape

    # [B, L, H, E] -> [BH, L, E]
    def shard(x):
        return np.ascontiguousarray(x.transpose(0, 2, 1, 3).reshape(BH, L, E))

    qs, ks, vs = shard(q), shard(k), shard(v)
    in_maps = [
        {
            "q": qs[c * BH_PER_CORE : (c + 1) * BH_PER_CORE],
            "k": ks[c * BH_PER_CORE : (c + 1) * BH_PER_CORE],
            "v": vs[c * BH_PER_CORE : (c + 1) * BH_PER_CORE],
        }
        for c in range(NCORES)
    ]
    nc = _get_program()
    res = run_bass_kernel_spmd(nc, in_maps, core_ids=list(range(NCORES)))
    o = np.concatenate([res.results[c]["o"] for c in range(NCORES)], axis=0)
    # [BH, L, E] -> [B, L, H, E]
    return np.ascontiguousarray(
        o.reshape(B, H, L, E).transpose(0, 2, 1, 3)
    ).astype(np.float32)


if __name__ == "__main__":
    rng = np.random.default_rng(0)
    qq = rng.standard_normal((B, L, H, E), dtype=np.float32)
    kk = rng.standard_normal((B, L, H, E), dtype=np.float32)
    vv = rng.standard_normal((B, L, H, E), dtype=np.float32)
    out = kernel(queries=qq, keys=kk, values=vv)
    print(out.shape, out.dtype)
